# revision 26
# baseline (speedup 1.0000x reference)
"""ConvLSTM3D encoder kernel for 8 trn2 NeuronCores.

Sharding: core c in [0,8) handles batch b = c//4, z-slab k = c%4 (8 output
planes z in [8k, 8k+8)).  The sequential T=10 loop runs on-device; per-step
halo exchange (1 plane each side of the slab) goes through an AllGather over
all 8 cores (shared-output collective, bf16 payload).

Conv mapping: gates = Wx (x) x_t (stride 2) + Wh (x) h + b is computed as a
single K=128 matmul accumulation stream per 512-voxel output chunk:
  partitions  0..95  : three z-shifted copies of h (dz = 0,1,2), bf16
  partitions 96..122 : host-precomputed im2col taps of x_t (27 taps), bf16
  partition  123     : ones (bias row)
For each (dy,dx) in 3x3, one standalone LDWEIGHTS + four non-self-loading
matmuls with an AP offset of (dy,dx) into the padded (34x34) plane layout
contract channels x dz at once; the x-conv and bias blocks ride along in
the delta=(0,0) group only (their lhsT rows are zero in the other eight).
PSUM accumulation and the LSTM pointwise (incl. the carried cell state c)
are fp32; only the h that feeds the next matmul (and the halo exchange) is
rounded to bf16.

Scheduling: 4 psum tiles per step covering plane pairs (3,4),(1,2),(5,6),
(0,7) in that order; the pairing keeps planes consecutive (fusing h
broadcast copies into 64-partition ops) and ensures each tile of the NEXT
step only needs h planes produced early enough, with the halo-dependent
pair (0,7) last.  Pointwise runs as one [128,1024] group for tiles 0+1 and
per-tile [64,1024] groups for tiles 2,3; emission is phase-split
(mul/mul/add/tanh fronts vs mulh+copies tails, interleaved) so ready work
never queues behind cross-engine stalls.  The AllGather fires right after
tile (0,7)'s h and hides under the next step's interior tiles.
"""

import os
import sys
from contextlib import ExitStack

import numpy as np

for _p in ("/opt/trn_rl_repo", "/root/.axon_site/_ro/trn_rl_repo"):
    if os.path.isdir(_p) and _p not in sys.path:
        sys.path.insert(0, _p)

import concourse.bass as bass
import concourse.bacc as bacc
import concourse.mybir as mybir
from concourse import tile
from concourse.bass_utils import run_bass_kernel_spmd

F32 = mybir.dt.float32
I32 = mybir.dt.int32
BF16 = mybir.dt.bfloat16

T = 10
CH = 32          # hidden channels
SLAB = 8         # output planes per core
PLW = 34         # padded plane width
PL = PLW * PLW   # padded plane elements (1156)
HS_FREE = SLAB * PL  # h-stack free size per partition (9248)
DELTAS = [(dy, dx) for dy in range(3) for dx in range(3)]
# plane pairs per psum tile, in processing order (see module docstring)
PAIRS = [(3, 4), (1, 2), (5, 6), (0, 7)]
RG = [[0, 1, 2, 3, 4, 5, 6, 7]]

_prog_cache = {}


def _build_program(nsteps=T, halo=True, bcast=True, pw=True):
    key = (nsteps, halo, bcast, pw)
    if key in _prog_cache:
        return _prog_cache[key]

    nc = bacc.Bacc(num_devices=8)

    xim_d = nc.dram_tensor("xim", [T, 28, HS_FREE], BF16, kind="ExternalInput")
    whl_d = nc.dram_tensor("whl", [9, 128, 128], BF16, kind="ExternalInput")
    hoff_d = nc.dram_tensor("hoff", [1, 2], I32, kind="ExternalInput")
    hout_d = nc.dram_tensor("hout", [CH, SLAB, 32, 32], F32, kind="ExternalOutput")
    agin = nc.dram_tensor("agin", [3, CH, 1024], BF16)
    agout = nc.dram_tensor("agout", [24, CH, 1024], BF16, addr_space="Shared")

    with ExitStack() as ctx:
        tc = ctx.enter_context(tile.TileContext(nc))
        pers = ctx.enter_context(tc.tile_pool(name="pers", bufs=1))
        psum = ctx.enter_context(tc.tile_pool(name="psum", bufs=2, space="PSUM"))
        work = ctx.enter_context(tc.tile_pool(name="work", bufs=2))

        hstack = [
            pers.tile([128, HS_FREE], BF16, tag="hstackA", name="hstackA"),
            pers.tile([128, HS_FREE], BF16, tag="hstackB", name="hstackB"),
        ]
        wh_sb = pers.tile([128, 9 * 128], BF16, tag="wh")
        gates = pers.tile([128, 2 * 2048], F32, tag="gates")
        c_state_p = pers.tile([128, 1024], F32, tag="cstatep")
        c_state_s = pers.tile([64, 2048], F32, tag="cstates")
        zscr = pers.tile([32, 1024], BF16, tag="zscr")

        # ---- init (on-chip zero fill; no HBM zero traffic) ----
        nc.vector.memset(hstack[0][:, :].bitcast(F32), 0.0)
        nc.vector.memset(hstack[1][:, :].bitcast(F32), 0.0)
        nc.vector.memset(c_state_p[:, :], 0.0)
        nc.vector.memset(c_state_s[:, :], 0.0)
        nc.vector.memset(zscr[:, :].bitcast(F32), 0.0)
        nc.sync.dma_start(out=agin[2], in_=zscr[:, :])
        for _d in range(9):
            nc.sync.dma_start(out=wh_sb[:, 128 * _d:128 * (_d + 1)],
                              in_=whl_d[_d])
        nc.sync.dma_start(out=hstack[0][96:124, :], in_=xim_d[0])

        r_lo = nc.alloc_register(mybir.EngineType.Pool, "r_lo")
        r_hi = nc.alloc_register(mybir.EngineType.Pool, "r_hi")
        nc.reg_load(r_lo, hoff_d[0:1, 0:1])
        nc.reg_load(r_hi, hoff_d[0:1, 1:2])
        rv_lo = nc.snap(r_lo, min_val=0, max_val=23)
        rv_hi = nc.snap(r_hi, min_val=0, max_val=23)

        hsv = [h[:, :].rearrange("p (z y x) -> p z y x", z=SLAB, y=PLW, x=PLW)
               for h in hstack]

        # engine rotation for h broadcast copies (vector 2x, scalar 1x);
        # gpsimd only gets the two latest-consumed tail copies
        bcast_engines = [nc.vector, nc.vector, nc.scalar]
        eng_state = [0]

        def bc_copy(eng, dst, src):
            if eng is nc.scalar:
                eng.copy(dst, src)
            else:
                eng.tensor_copy(dst, src)

        def bc_rot(dst, src):
            eng = bcast_engines[eng_state[0] % len(bcast_engines)]
            eng_state[0] += 1
            bc_copy(eng, dst, src)

        def mm_noload(out, lhsT, rhs, start, stop):
            """matmul that reuses the PE array's already-loaded stationary
            weights (ldweights=False); paired with a standalone ldweights
            per delta to drop the per-matmul reload (ldw-opt is off)."""
            eng = nc.tensor
            ifmap_ap = eng.lower_ap(rhs.opt({0}), opt=False)
            weights_ap = eng.lower_ap(lhsT.opt({0}), opt=False,
                                      for_matmul_weights=True)
            out_ap = eng.lower_ap(out)
            return eng.add_instruction(
                mybir.InstMatmult(
                    name=eng.bass.get_next_instruction_name(),
                    replication_resolution=0,
                    replication_shift_amnt=0,
                    replication_num_rows=0,
                    start_tensor_calc=start,
                    stop_tensor_calc=stop,
                    ins=[ifmap_ap, weights_ap],
                    outs=[out_ap],
                    perf_mode=None,
                    is_transpose=None,
                    ifmap_quant_offset=None,
                    weights_quant_offset=None,
                    bass_skip_group_check=True,
                    ldweights=False,
                    tile_position=(0, 0),
                    tile_size=(128, 128),
                ))

        def emit_tile_mm(x, curv, t, gt, qoff):
            """matmuls + activation + gate regroup for plane pair x; the
            regroup lands in quarters qoff..qoff+1 of gt"""
            ps = psum.tile([128, 2048], F32, tag="ps", name="ps")
            if t == 0:
                for cq in range(4):
                    pl, hf = PAIRS[x][cq // 2], cq % 2
                    r0 = 16 * hf
                    rhs = curv[:, pl, r0:r0 + 16, 0:32]
                    nc.tensor.matmul(ps[:, 512 * cq:512 * (cq + 1)],
                                     lhsT=wh_sb[:, 0:128],
                                     rhs=rhs, start=True, stop=True)
            else:
                for di, (dy, dx) in enumerate(DELTAS):
                    lhsT = wh_sb[:, 128 * di:128 * (di + 1)]
                    nc.tensor.ldweights(lhsT)
                    for cq in range(4):
                        pl, hf = PAIRS[x][cq // 2], cq % 2
                        r0 = 16 * hf
                        rhs = curv[:, pl, r0 + dy:r0 + dy + 16, dx:dx + 32]
                        mm_noload(
                            ps[:, 512 * cq:512 * (cq + 1)],
                            lhsT, rhs,
                            start=(di == 0), stop=(di == 8))
            span = slice((x % 2) * 2048, (x % 2) * 2048 + 2048)
            nc.scalar.activation(gates[0:96, span], ps[0:96, :],
                                 mybir.ActivationFunctionType.Sigmoid)
            nc.scalar.activation(gates[96:128, span], ps[96:128, :],
                                 mybir.ActivationFunctionType.Tanh)
            for G in range(4):
                for q in range(2):
                    qq = qoff + q
                    nc.sync.dma_start(
                        out=gt[G][32 * qq:32 * qq + 32, :],
                        in_=gates[32 * G:32 * G + 32,
                                  (x % 2) * 2048 + 1024 * q:
                                  (x % 2) * 2048 + 1024 * q + 1024])

        # pointwise groups: group 0 = tiles 0+1 (planes 3,4,1,2) at
        # [128,1024]; groups 2,3 = those tiles alone at [64,1024]
        pwst = {}

        def pw_front(g, gts, t):
            if not pw:
                return
            last = t == nsteps - 1
            P = 128 if g == 0 else 64
            c_sl = (c_state_p[:, :] if g == 0
                    else c_state_s[:, 1024 * (g - 2):1024 * (g - 1)])
            prod = work.tile([P, 1024], F32, tag="prod0" if g == 0 else "prods")
            tmp = work.tile([P, 1024], F32, tag="tmp0" if g == 0 else "tmps")
            tanhc = work.tile([P, 1024], F32, tag="tanhc0" if g == 0 else "tanhcs")
            h_t = work.tile([P, 1024], F32 if last else BF16, tag="ht0" if g == 0 else "hts")
            i_t, f_t, o_t, g_t = gts
            nc.vector.tensor_mul(prod[:, :], i_t[:, :], g_t[:, :])
            nc.vector.tensor_mul(tmp[:, :], f_t[:, :], c_sl)
            nc.vector.tensor_add(c_sl, prod[:, :], tmp[:, :])
            nc.scalar.activation(tanhc[:, :], c_sl,
                                 mybir.ActivationFunctionType.Tanh)
            pwst[g] = (o_t, tanhc, h_t)

        def pw_mulh(g):
            if not pw:
                return
            o_t, tanhc, h_t = pwst[g]
            nc.vector.tensor_mul(h_t[:, :], o_t[:, :], tanhc[:, :])

        def pw_copies(g, nxtv, t):
            if not pw:
                return
            last = t == nsteps - 1
            _, _, h_t = pwst[g]
            planes = (3, 4, 1, 2) if g == 0 else PAIRS[g]
            if last:
                for q, pl in enumerate(planes):
                    src3 = h_t[32 * q:32 * q + 32, :].rearrange(
                        "p (y x) -> p y x", y=32, x=32)
                    nc.sync.dma_start(out=hout_d[:, pl, :, :], in_=src3)
                return
            if not bcast:
                return
            h3 = h_t[:, :].rearrange("p (y x) -> p y x", y=32, x=32)
            for q in range(0, len(planes), 2):
                A, B = planes[q], planes[q + 1]
                hA = h3[32 * q:32 * q + 32]
                hB = h3[32 * q + 32:32 * q + 64]
                if B == A + 1:
                    # fused: [g0<-hA, g1<-hB] at plane A+1 (one 64-high op;
                    # [32:96] would cross the quadrant rule -> singles)
                    bc_rot(nxtv[0:64, A + 1, 1:33, 1:33],
                           h3[32 * q:32 * q + 64])
                    bc_rot(nxtv[32:64, A, 1:33, 1:33], hA)
                    bc_rot(nxtv[64:96, A, 1:33, 1:33], hB)
                    if A - 1 >= 0:
                        bc_rot(nxtv[64:96, A - 1, 1:33, 1:33], hA)
                    if B + 1 <= 7:
                        bc_rot(nxtv[0:32, B + 1, 1:33, 1:33], hB)
                else:
                    # planes (0,7): slots only the next (0,7) tile consumes
                    # go to gpsimd (idle right after the collective)
                    bc_copy(nc.gpsimd, nxtv[32:64, 0, 1:33, 1:33], hA)
                    bc_rot(nxtv[0:32, 1, 1:33, 1:33], hA)
                    bc_copy(nc.gpsimd, nxtv[32:64, 7, 1:33, 1:33], hB)
                    bc_rot(nxtv[64:96, 6, 1:33, 1:33], hB)

        def fire_allgather(nxtv, t):
            if not pw or t == nsteps - 1:
                return
            _, _, h_t = pwst[3]
            nc.sync.dma_start(out=agin[0], in_=h_t[0:32, :])
            nc.sync.dma_start(out=agin[1], in_=h_t[32:64, :])
            if halo:
                nc.gpsimd.collective_compute(
                    "AllGather", mybir.AluOpType.bypass, replica_groups=RG,
                    ins=[agin[:, :, :]], outs=[agout[:, :, :]])
                halo_lo = agout[bass.ds(rv_lo, 1)].squeeze(0).rearrange(
                    "c (y x) -> c y x", y=32, x=32)
                halo_hi = agout[bass.ds(rv_hi, 1)].squeeze(0).rearrange(
                    "c (y x) -> c y x", y=32, x=32)
                nc.gpsimd.dma_start(out=nxtv[0:32, 0, 1:33, 1:33],
                                    in_=halo_lo)
                nc.gpsimd.dma_start(out=nxtv[64:96, 7, 1:33, 1:33],
                                    in_=halo_hi)

        T_ = nsteps
        for t in range(T_):
            curv, nxtv = hsv[t % 2], hsv[(t + 1) % 2]
            nxt = hstack[(t + 1) % 2]
            if t + 1 < T_:
                nc.sync.dma_start(out=nxt[96:124, :], in_=xim_d[t + 1])

            gtp = [work.tile([128, 1024], F32, tag=f"gp{G}", name=f"gp{G}")
                   for G in range(4)]
            gt2 = [work.tile([64, 1024], F32, tag=f"gs{G}", name=f"gs{G}")
                   for G in range(4)]
            gt3 = None  # allocated after tile 2's pointwise consumes gt2
            for x in range(4):
                if x == 2:
                    pw_front(0, gtp, t)
                if x == 3:
                    pw_mulh(0)
                    pw_front(2, gt2, t)
                    gt3 = [work.tile([64, 1024], F32, tag=f"gs{G}",
                                     name=f"gs{G}b") for G in range(4)]
                dst, qoff = ((gtp, 2 * x) if x < 2 else
                             ((gt2, 0) if x == 2 else (gt3, 0)))
                emit_tile_mm(x, curv, t, dst, qoff)
            pw_copies(0, nxtv, t)
            pw_mulh(2)
            pw_copies(2, nxtv, t)
            pw_front(3, gt3, t)
            pw_mulh(3)
            fire_allgather(nxtv, t)
            pw_copies(3, nxtv, t)

    nc.finalize()
    _prog_cache[key] = nc
    return nc


def _host_inputs(input_batch, Wx, Wh, b):
    input_batch = np.asarray(input_batch, dtype=np.float32)
    Wx = np.asarray(Wx, dtype=np.float32)
    Wh = np.asarray(Wh, dtype=np.float32)
    b = np.asarray(b, dtype=np.float32)

    import ml_dtypes

    def to_bf16(a):
        return np.asarray(a, np.float32).astype(ml_dtypes.bfloat16)

    xp = np.zeros((2, T, 66, 66, 66), np.float32)
    xp[:, :, 1:65, 1:65, 1:65] = input_batch[:, :, 0]

    whl = np.zeros((9, 128, 128), np.float32)
    for di, (dy, dx) in enumerate(DELTAS):
        for g in range(3):
            whl[di, 32 * g:32 * g + 32, :] = Wh[:, :, g, dy, dx].T
    whl[0, 96:123, :] = Wx[:, 0].reshape(128, 27).T
    whl[0, 123, :] = b

    in_maps = []
    for c in range(8):
        bidx, k = divmod(c, 4)
        z0 = 8 * k
        xim = np.zeros((T, 28, SLAB, PLW, PLW), np.float32)
        for tz in range(3):
            for ty in range(3):
                for tx in range(3):
                    tap = tz * 9 + ty * 3 + tx
                    xim[:, tap, :, 0:32, 0:32] = xp[
                        bidx, :, 2 * z0 + tz:2 * z0 + tz + 16:2,
                        ty:ty + 64:2, tx:tx + 64:2]
        xim[:, 27, :, 0:32, 0:32] = 1.0
        lo_slot = c * 3 + 2 if k == 0 else (c - 1) * 3 + 1
        hi_slot = c * 3 + 2 if k == 3 else (c + 1) * 3 + 0
        in_maps.append({
            "xim": to_bf16(xim.reshape(T, 28, HS_FREE)),
            "whl": to_bf16(whl),
            "hoff": np.array([[lo_slot, hi_slot]], np.int32),
        })
    return in_maps


def run_cores(in_maps, nsteps=T, halo=True, bcast=True, pw=True, **kwargs):
    nc = _build_program(nsteps, halo, bcast, pw)
    return run_bass_kernel_spmd(nc, in_maps, list(range(8)), **kwargs)


def kernel(input_batch, Wx, Wh, b):
    in_maps = _host_inputs(input_batch, Wx, Wh, b)
    res = run_cores(in_maps)
    out = np.zeros((2, CH, 32, 32, 32), np.float32)
    for c in range(8):
        bidx, k = divmod(c, 4)
        out[bidx, :, 8 * k:8 * k + 8] = res.results[c]["hout"]
    return out


# revision 27
# speedup vs baseline: 1.0132x; 1.0132x over previous
"""ConvLSTM3D encoder kernel for 8 trn2 NeuronCores.

Sharding: core c in [0,8) handles batch b = c//4, z-slab k = c%4 (8 output
planes z in [8k, 8k+8)).  The sequential T=10 loop runs on-device; per-step
halo exchange (1 plane each side of the slab) goes through an AllGather over
all 8 cores (shared-output collective, bf16 payload).

Conv mapping: gates = Wx (x) x_t (stride 2) + Wh (x) h + b is computed as a
single K=128 matmul accumulation stream per 512-voxel output chunk:
  partitions  0..95  : three z-shifted copies of h (dz = 0,1,2), bf16
  partitions 96..122 : host-precomputed im2col taps of x_t (27 taps), bf16
  partition  123     : ones (bias row)
For each (dy,dx) in 3x3, one standalone LDWEIGHTS + four non-self-loading
matmuls with an AP offset of (dy,dx) into the padded (34x34) plane layout
contract channels x dz at once; the x-conv and bias blocks ride along in
the delta=(0,0) group only (their lhsT rows are zero in the other eight).
PSUM accumulation and the LSTM pointwise (incl. the carried cell state c)
are fp32; only the h that feeds the next matmul (and the halo exchange) is
rounded to bf16.

Scheduling: 4 psum tiles per step covering plane pairs (3,4),(1,2),(5,6),
(0,7) in that order; the pairing keeps planes consecutive (so two of the
three h broadcast copies fuse into one 64-partition op) and ensures each
tile of the NEXT step only needs h planes produced early enough, with the
halo-dependent pair (0,7) last.  LSTM pointwise+broadcast for tile x is
emitted while tile x+1's matmuls stream; the AllGather fires right after
tile (0,7)'s pointwise and hides under the next step's interior tiles.
"""

import os
import sys
from contextlib import ExitStack

import numpy as np

for _p in ("/opt/trn_rl_repo", "/root/.axon_site/_ro/trn_rl_repo"):
    if os.path.isdir(_p) and _p not in sys.path:
        sys.path.insert(0, _p)

import concourse.bass as bass
import concourse.bacc as bacc
import concourse.mybir as mybir
from concourse import tile
from concourse.bass_utils import run_bass_kernel_spmd

F32 = mybir.dt.float32
I32 = mybir.dt.int32
BF16 = mybir.dt.bfloat16

T = 10
CH = 32          # hidden channels
SLAB = 8         # output planes per core
PLW = 34         # padded plane width
PL = PLW * PLW   # padded plane elements (1156)
HS_FREE = SLAB * PL  # h-stack free size per partition (9248)
DELTAS = [(dy, dx) for dy in range(3) for dx in range(3)]
# plane pairs per psum tile, in processing order (see module docstring)
PAIRS = [(3, 4), (1, 2), (5, 6), (0, 7)]
RG = [[0, 1, 2, 3, 4, 5, 6, 7]]

_prog_cache = {}


def _build_program(nsteps=T, halo=True, bcast=True, pw=True):
    key = (nsteps, halo, bcast, pw)
    if key in _prog_cache:
        return _prog_cache[key]

    nc = bacc.Bacc(num_devices=8)

    xim_d = nc.dram_tensor("xim", [T, 28, HS_FREE], BF16, kind="ExternalInput")
    whl_d = nc.dram_tensor("whl", [9, 128, 128], BF16, kind="ExternalInput")
    hoff_d = nc.dram_tensor("hoff", [1, 2], I32, kind="ExternalInput")
    hout_d = nc.dram_tensor("hout", [CH, SLAB, 32, 32], F32, kind="ExternalOutput")
    agin = nc.dram_tensor("agin", [3, CH, 1024], BF16)
    agout = nc.dram_tensor("agout", [24, CH, 1024], BF16, addr_space="Shared")

    with ExitStack() as ctx:
        tc = ctx.enter_context(tile.TileContext(nc))
        pers = ctx.enter_context(tc.tile_pool(name="pers", bufs=1))
        psum = ctx.enter_context(tc.tile_pool(name="psum", bufs=2, space="PSUM"))
        work = ctx.enter_context(tc.tile_pool(name="work", bufs=2))

        hstack = [
            pers.tile([128, HS_FREE], BF16, tag="hstackA", name="hstackA"),
            pers.tile([128, HS_FREE], BF16, tag="hstackB", name="hstackB"),
        ]
        wh_sb = pers.tile([128, 9 * 128], BF16, tag="wh")
        gates = pers.tile([128, 16 * 512], F32, tag="gates")
        c_state = pers.tile([64, 4096], F32, tag="cstate")
        zscr = pers.tile([32, 1024], BF16, tag="zscr")

        # ---- init (on-chip zero fill; no HBM zero traffic) ----
        nc.vector.memset(hstack[0][:, :].bitcast(F32), 0.0)
        nc.vector.memset(hstack[1][:, :].bitcast(F32), 0.0)
        nc.vector.memset(c_state[:, :], 0.0)
        nc.vector.memset(zscr[:, :].bitcast(F32), 0.0)
        nc.sync.dma_start(out=agin[2], in_=zscr[:, :])
        for _d in range(9):
            nc.sync.dma_start(out=wh_sb[:, 128 * _d:128 * (_d + 1)],
                              in_=whl_d[_d])
        nc.sync.dma_start(out=hstack[0][96:124, :], in_=xim_d[0])

        r_lo = nc.alloc_register(mybir.EngineType.Pool, "r_lo")
        r_hi = nc.alloc_register(mybir.EngineType.Pool, "r_hi")
        nc.reg_load(r_lo, hoff_d[0:1, 0:1])
        nc.reg_load(r_hi, hoff_d[0:1, 1:2])
        rv_lo = nc.snap(r_lo, min_val=0, max_val=23)
        rv_hi = nc.snap(r_hi, min_val=0, max_val=23)

        hsv = [h[:, :].rearrange("p (z y x) -> p z y x", z=SLAB, y=PLW, x=PLW)
               for h in hstack]

        # engine rotation for h broadcast copies (vector 2x, scalar 1x);
        # gpsimd only gets the two latest-consumed tail copies
        bcast_engines = [nc.vector, nc.vector, nc.scalar]
        eng_state = [0]

        def bc_copy(eng, dst, src):
            if eng is nc.scalar:
                eng.copy(dst, src)
            else:
                eng.tensor_copy(dst, src)

        def bc_rot(dst, src):
            eng = bcast_engines[eng_state[0] % len(bcast_engines)]
            eng_state[0] += 1
            bc_copy(eng, dst, src)

        def mm_noload(out, lhsT, rhs, start, stop):
            """matmul that reuses the PE array's already-loaded stationary
            weights (ldweights=False); paired with a standalone ldweights
            per delta to drop the per-matmul reload (ldw-opt is off)."""
            eng = nc.tensor
            ifmap_ap = eng.lower_ap(rhs.opt({0}), opt=False)
            weights_ap = eng.lower_ap(lhsT.opt({0}), opt=False,
                                      for_matmul_weights=True)
            out_ap = eng.lower_ap(out)
            return eng.add_instruction(
                mybir.InstMatmult(
                    name=eng.bass.get_next_instruction_name(),
                    replication_resolution=0,
                    replication_shift_amnt=0,
                    replication_num_rows=0,
                    start_tensor_calc=start,
                    stop_tensor_calc=stop,
                    ins=[ifmap_ap, weights_ap],
                    outs=[out_ap],
                    perf_mode=None,
                    is_transpose=None,
                    ifmap_quant_offset=None,
                    weights_quant_offset=None,
                    bass_skip_group_check=True,
                    ldweights=False,
                    tile_position=(0, 0),
                    tile_size=(128, 128),
                ))

        def emit_tile_mm(x, curv, t):
            """matmuls + activation + gate regroup for plane pair x"""
            ps = psum.tile([128, 2048], F32, tag="ps", name="ps")
            if t == 0:
                for cq in range(4):
                    pl, hf = PAIRS[x][cq // 2], cq % 2
                    r0 = 16 * hf
                    rhs = curv[:, pl, r0:r0 + 16, 0:32]
                    nc.tensor.matmul(ps[:, 512 * cq:512 * (cq + 1)],
                                     lhsT=wh_sb[:, 0:128],
                                     rhs=rhs, start=True, stop=True)
            else:
                for di, (dy, dx) in enumerate(DELTAS):
                    lhsT = wh_sb[:, 128 * di:128 * (di + 1)]
                    nc.tensor.ldweights(lhsT)
                    for cq in range(4):
                        pl, hf = PAIRS[x][cq // 2], cq % 2
                        r0 = 16 * hf
                        rhs = curv[:, pl, r0 + dy:r0 + dy + 16, dx:dx + 32]
                        mm_noload(
                            ps[:, 512 * cq:512 * (cq + 1)],
                            lhsT, rhs,
                            start=(di == 0), stop=(di == 8))
            span = slice(4 * x * 512, (4 * x + 4) * 512)
            nc.scalar.activation(gates[0:96, span], ps[0:96, :],
                                 mybir.ActivationFunctionType.Sigmoid)
            nc.scalar.activation(gates[96:128, span], ps[96:128, :],
                                 mybir.ActivationFunctionType.Tanh)
            gt = [work.tile([64, 1024], F32, tag=f"gate{G}", name=f"gate{G}")
                  for G in range(4)]
            for G in range(4):
                for q in range(2):
                    nc.sync.dma_start(
                        out=gt[G][32 * q:32 * q + 32, :],
                        in_=gates[32 * G:32 * G + 32,
                                  (4 * x + 2 * q) * 512:
                                  (4 * x + 2 * q + 2) * 512])
            return gt

        def emit_tile_pw(x, gt, nxtv, t):
            """LSTM pointwise + h distribution for plane pair x"""
            if not pw:
                return
            last = t == nsteps - 1
            i_t, f_t, o_t, g_t = gt
            prod = work.tile([64, 1024], F32, tag="prod")
            tmp = work.tile([64, 1024], F32, tag="tmp")
            tanhc = work.tile([64, 1024], F32, tag="tanhc")
            h_t = work.tile([64, 1024], F32 if last else BF16, tag="ht")
            c_sl = c_state[:, 1024 * x:1024 * x + 1024]
            nc.vector.tensor_mul(prod[:, :], i_t[:, :], g_t[:, :])
            nc.vector.tensor_mul(tmp[:, :], f_t[:, :], c_sl)
            nc.vector.tensor_add(c_sl, prod[:, :], tmp[:, :])
            nc.scalar.activation(tanhc[:, :], c_sl,
                                 mybir.ActivationFunctionType.Tanh)
            nc.vector.tensor_mul(h_t[:, :], o_t[:, :], tanhc[:, :])

            A, B = PAIRS[x]
            if last:
                for q, pl in ((0, A), (1, B)):
                    src3 = h_t[32 * q:32 * q + 32, :].rearrange(
                        "p (y x) -> p y x", y=32, x=32)
                    nc.sync.dma_start(out=hout_d[:, pl, :, :], in_=src3)
                return

            if x == 3:
                # h for planes 0 and 7 -> collective input, then fire the
                # AllGather; its wire time hides under the next step's
                # interior-tile matmuls
                nc.sync.dma_start(out=agin[0], in_=h_t[0:32, :])
                nc.sync.dma_start(out=agin[1], in_=h_t[32:64, :])
                if halo:
                    nc.gpsimd.collective_compute(
                        "AllGather", mybir.AluOpType.bypass, replica_groups=RG,
                        ins=[agin[:, :, :]], outs=[agout[:, :, :]])
                    halo_lo = agout[bass.ds(rv_lo, 1)].squeeze(0).rearrange(
                        "c (y x) -> c y x", y=32, x=32)
                    halo_hi = agout[bass.ds(rv_hi, 1)].squeeze(0).rearrange(
                        "c (y x) -> c y x", y=32, x=32)
                    nc.gpsimd.dma_start(out=nxtv[0:32, 0, 1:33, 1:33],
                                        in_=halo_lo)
                    nc.gpsimd.dma_start(out=nxtv[64:96, 7, 1:33, 1:33],
                                        in_=halo_hi)

            if not bcast:
                return
            h3 = h_t[:, :].rearrange("p (y x) -> p y x", y=32, x=32)
            if B == A + 1:
                # fused broadcasts: [g0<-hA, g1<-hB] at plane A+1, one
                # 64-partition op; [32:96] in one op would cross a quadrant
                # boundary at base 32 (illegal) -> two singles
                bc_rot(nxtv[0:64, A + 1, 1:33, 1:33], h3[0:64])
                bc_rot(nxtv[32:64, A, 1:33, 1:33], h3[0:32])
                bc_rot(nxtv[64:96, A, 1:33, 1:33], h3[32:64])
                if A - 1 >= 0:
                    bc_rot(nxtv[64:96, A - 1, 1:33, 1:33], h3[0:32])
                if B + 1 <= 7:
                    bc_rot(nxtv[0:32, B + 1, 1:33, 1:33], h3[32:64])
            else:
                # pair (0,7): four singles; the two slots only consumed by
                # the next step's (0,7) tile go to gpsimd (it is idle after
                # the collective completes, which is exactly their window)
                bc_copy(nc.gpsimd, nxtv[32:64, 0, 1:33, 1:33], h3[0:32])
                bc_rot(nxtv[0:32, 1, 1:33, 1:33], h3[0:32])
                bc_copy(nc.gpsimd, nxtv[32:64, 7, 1:33, 1:33], h3[32:64])
                bc_rot(nxtv[64:96, 6, 1:33, 1:33], h3[32:64])

        T_ = nsteps
        for t in range(T_):
            curv, nxtv = hsv[t % 2], hsv[(t + 1) % 2]
            nxt = hstack[(t + 1) % 2]
            if t + 1 < T_:
                nc.sync.dma_start(out=nxt[96:124, :], in_=xim_d[t + 1])

            gts = [None] * 4
            for x in range(4):
                if x >= 1:
                    emit_tile_pw(x - 1, gts[x - 1], nxtv, t)
                gts[x] = emit_tile_mm(x, curv, t)
            emit_tile_pw(3, gts[3], nxtv, t)

    nc.finalize()
    _prog_cache[key] = nc
    return nc


def _host_inputs(input_batch, Wx, Wh, b):
    input_batch = np.asarray(input_batch, dtype=np.float32)
    Wx = np.asarray(Wx, dtype=np.float32)
    Wh = np.asarray(Wh, dtype=np.float32)
    b = np.asarray(b, dtype=np.float32)

    import ml_dtypes

    def to_bf16(a):
        return np.asarray(a, np.float32).astype(ml_dtypes.bfloat16)

    xp = np.zeros((2, T, 66, 66, 66), np.float32)
    xp[:, :, 1:65, 1:65, 1:65] = input_batch[:, :, 0]

    whl = np.zeros((9, 128, 128), np.float32)
    for di, (dy, dx) in enumerate(DELTAS):
        for g in range(3):
            whl[di, 32 * g:32 * g + 32, :] = Wh[:, :, g, dy, dx].T
    whl[0, 96:123, :] = Wx[:, 0].reshape(128, 27).T
    whl[0, 123, :] = b

    in_maps = []
    for c in range(8):
        bidx, k = divmod(c, 4)
        z0 = 8 * k
        xim = np.zeros((T, 28, SLAB, PLW, PLW), np.float32)
        for tz in range(3):
            for ty in range(3):
                for tx in range(3):
                    tap = tz * 9 + ty * 3 + tx
                    xim[:, tap, :, 0:32, 0:32] = xp[
                        bidx, :, 2 * z0 + tz:2 * z0 + tz + 16:2,
                        ty:ty + 64:2, tx:tx + 64:2]
        xim[:, 27, :, 0:32, 0:32] = 1.0
        lo_slot = c * 3 + 2 if k == 0 else (c - 1) * 3 + 1
        hi_slot = c * 3 + 2 if k == 3 else (c + 1) * 3 + 0
        in_maps.append({
            "xim": to_bf16(xim.reshape(T, 28, HS_FREE)),
            "whl": to_bf16(whl),
            "hoff": np.array([[lo_slot, hi_slot]], np.int32),
        })
    return in_maps


def run_cores(in_maps, nsteps=T, halo=True, bcast=True, pw=True, **kwargs):
    nc = _build_program(nsteps, halo, bcast, pw)
    return run_bass_kernel_spmd(nc, in_maps, list(range(8)), **kwargs)


def kernel(input_batch, Wx, Wh, b):
    in_maps = _host_inputs(input_batch, Wx, Wh, b)
    res = run_cores(in_maps)
    out = np.zeros((2, CH, 32, 32, 32), np.float32)
    for c in range(8):
        bidx, k = divmod(c, 4)
        out[bidx, :, 8 * k:8 * k + 8] = res.results[c]["hout"]
    return out


# revision 29
# speedup vs baseline: 1.0241x; 1.0107x over previous
"""ConvLSTM3D encoder kernel for 8 trn2 NeuronCores.

Sharding: core c in [0,8) handles batch b = c//4, z-slab k = c%4 (8 output
planes z in [8k, 8k+8)).  The sequential T=10 loop runs on-device; per-step
halo exchange (1 plane each side of the slab) goes through an AllGather over
all 8 cores (shared-output collective, bf16 payload).

Conv mapping: gates = Wx (x) x_t (stride 2) + Wh (x) h + b is computed as a
single K=128 matmul accumulation stream per 512-voxel output chunk:
  partitions  0..95  : three z-shifted copies of h (dz = 0,1,2), bf16
  partitions 96..122 : host-precomputed im2col taps of x_t (27 taps), bf16
  partition  123     : ones (bias row)
For each (dy,dx) in 3x3, four 512-wide matmuls with an AP offset of
(dy,dx) into the padded (34x34) plane layout contract channels x dz at
once; the x-conv and bias blocks ride along in the delta=(0,0) group only
(their lhsT rows are zero in the other eight).
PSUM accumulation and the LSTM pointwise (incl. the carried cell state c)
are fp32; only the h that feeds the next matmul (and the halo exchange) is
rounded to bf16.

Scheduling: 4 psum tiles per step covering plane pairs (3,4),(1,2),(5,6),
(0,7) in that order; the pairing keeps planes consecutive (so two of the
three h broadcast copies fuse into one 64-partition op) and ensures each
tile of the NEXT step only needs h planes produced early enough, with the
halo-dependent pair (0,7) last.  LSTM pointwise+broadcast for tile x is
emitted while tile x+1's matmuls stream; the AllGather fires right after
tile (0,7)'s pointwise and hides under the next step's interior tiles.
"""

import os
import sys
from contextlib import ExitStack

import numpy as np

for _p in ("/opt/trn_rl_repo", "/root/.axon_site/_ro/trn_rl_repo"):
    if os.path.isdir(_p) and _p not in sys.path:
        sys.path.insert(0, _p)

import concourse.bass as bass
import concourse.bacc as bacc
import concourse.mybir as mybir
from concourse import tile
from concourse.bass_utils import run_bass_kernel_spmd

F32 = mybir.dt.float32
I32 = mybir.dt.int32
BF16 = mybir.dt.bfloat16

T = 10
CH = 32          # hidden channels
SLAB = 8         # output planes per core
PLW = 34         # padded plane width
PL = PLW * PLW   # padded plane elements (1156)
HS_FREE = SLAB * PL  # h-stack free size per partition (9248)
DELTAS = [(dy, dx) for dy in range(3) for dx in range(3)]
# plane pairs per psum tile, in processing order (see module docstring)
PAIRS = [(3, 4), (1, 2), (5, 6), (0, 7)]
RG = [[0, 1, 2, 3, 4, 5, 6, 7]]

_prog_cache = {}


def _build_program(nsteps=T, halo=True, bcast=True, pw=True):
    key = (nsteps, halo, bcast, pw)
    if key in _prog_cache:
        return _prog_cache[key]

    nc = bacc.Bacc(num_devices=8)

    xim_d = nc.dram_tensor("xim", [T, 28, HS_FREE], BF16, kind="ExternalInput")
    whl_d = nc.dram_tensor("whl", [9, 128, 128], BF16, kind="ExternalInput")
    hoff_d = nc.dram_tensor("hoff", [1, 2], I32, kind="ExternalInput")
    hout_d = nc.dram_tensor("hout", [CH, SLAB, 32, 32], F32, kind="ExternalOutput")
    agin = nc.dram_tensor("agin", [3, CH, 1024], BF16)
    agout = nc.dram_tensor("agout", [24, CH, 1024], BF16, addr_space="Shared")

    with ExitStack() as ctx:
        tc = ctx.enter_context(tile.TileContext(nc))
        pers = ctx.enter_context(tc.tile_pool(name="pers", bufs=1))
        psum = ctx.enter_context(tc.tile_pool(name="psum", bufs=2, space="PSUM"))
        work = ctx.enter_context(tc.tile_pool(name="work", bufs=2))

        hstack = [
            pers.tile([128, HS_FREE], BF16, tag="hstackA", name="hstackA"),
            pers.tile([128, HS_FREE], BF16, tag="hstackB", name="hstackB"),
        ]
        wh_sb = pers.tile([128, 9 * 128], BF16, tag="wh")
        gates = pers.tile([128, 16 * 512], F32, tag="gates")
        c_state = pers.tile([64, 4096], F32, tag="cstate")
        zscr = pers.tile([32, 1024], BF16, tag="zscr")

        # ---- init (on-chip zero fill; no HBM zero traffic) ----
        nc.vector.memset(hstack[0][:, :].bitcast(F32), 0.0)
        nc.vector.memset(hstack[1][:, :].bitcast(F32), 0.0)
        nc.vector.memset(c_state[:, :], 0.0)
        nc.vector.memset(zscr[:, :].bitcast(F32), 0.0)
        nc.sync.dma_start(out=agin[2], in_=zscr[:, :])
        for _d in range(9):
            nc.sync.dma_start(out=wh_sb[:, 128 * _d:128 * (_d + 1)],
                              in_=whl_d[_d])
        nc.sync.dma_start(out=hstack[0][96:124, :], in_=xim_d[0])

        r_lo = nc.alloc_register(mybir.EngineType.Pool, "r_lo")
        r_hi = nc.alloc_register(mybir.EngineType.Pool, "r_hi")
        nc.reg_load(r_lo, hoff_d[0:1, 0:1])
        nc.reg_load(r_hi, hoff_d[0:1, 1:2])
        rv_lo = nc.snap(r_lo, min_val=0, max_val=23)
        rv_hi = nc.snap(r_hi, min_val=0, max_val=23)

        hsv = [h[:, :].rearrange("p (z y x) -> p z y x", z=SLAB, y=PLW, x=PLW)
               for h in hstack]

        # engine rotation for h broadcast copies (vector 2x, scalar 1x);
        # gpsimd only gets the two latest-consumed tail copies
        bcast_engines = [nc.vector, nc.vector, nc.scalar]
        eng_state = [0]

        def bc_copy(eng, dst, src):
            if eng is nc.scalar:
                eng.copy(dst, src)
            else:
                eng.tensor_copy(dst, src)

        def bc_rot(dst, src):
            eng = bcast_engines[eng_state[0] % len(bcast_engines)]
            eng_state[0] += 1
            bc_copy(eng, dst, src)

        def emit_tile_mm(x, curv, t):
            """matmuls + activation + gate regroup for plane pair x"""
            ps = psum.tile([128, 2048], F32, tag="ps", name="ps")
            if t == 0:
                for cq in range(4):
                    pl, hf = PAIRS[x][cq // 2], cq % 2
                    r0 = 16 * hf
                    rhs = curv[:, pl, r0:r0 + 16, 0:32]
                    nc.tensor.matmul(ps[:, 512 * cq:512 * (cq + 1)],
                                     lhsT=wh_sb[:, 0:128],
                                     rhs=rhs, start=True, stop=True)
            else:
                for di, (dy, dx) in enumerate(DELTAS):
                    lhsT = wh_sb[:, 128 * di:128 * (di + 1)]
                    for cq in range(4):
                        pl, hf = PAIRS[x][cq // 2], cq % 2
                        r0 = 16 * hf
                        rhs = curv[:, pl, r0 + dy:r0 + dy + 16, dx:dx + 32]
                        nc.tensor.matmul(
                            ps[:, 512 * cq:512 * (cq + 1)],
                            lhsT=lhsT, rhs=rhs,
                            start=(di == 0), stop=(di == 8))
            span = slice(4 * x * 512, (4 * x + 4) * 512)
            nc.scalar.activation(gates[0:96, span], ps[0:96, :],
                                 mybir.ActivationFunctionType.Sigmoid)
            nc.scalar.activation(gates[96:128, span], ps[96:128, :],
                                 mybir.ActivationFunctionType.Tanh)
            gt = [work.tile([64, 1024], F32, tag=f"gate{G}", name=f"gate{G}")
                  for G in range(4)]
            for G in range(4):
                for q in range(2):
                    nc.sync.dma_start(
                        out=gt[G][32 * q:32 * q + 32, :],
                        in_=gates[32 * G:32 * G + 32,
                                  (4 * x + 2 * q) * 512:
                                  (4 * x + 2 * q + 2) * 512])
            return gt

        def emit_tile_pw(x, gt, nxtv, t):
            """LSTM pointwise + h distribution for plane pair x"""
            if not pw:
                return
            last = t == nsteps - 1
            i_t, f_t, o_t, g_t = gt
            prod = work.tile([64, 1024], F32, tag="prod")
            tmp = work.tile([64, 1024], F32, tag="tmp")
            tanhc = work.tile([64, 1024], F32, tag="tanhc")
            h_t = work.tile([64, 1024], F32 if last else BF16, tag="ht")
            c_sl = c_state[:, 1024 * x:1024 * x + 1024]
            nc.vector.tensor_mul(prod[:, :], i_t[:, :], g_t[:, :])
            nc.vector.tensor_mul(tmp[:, :], f_t[:, :], c_sl)
            nc.vector.tensor_add(c_sl, prod[:, :], tmp[:, :])
            nc.scalar.activation(tanhc[:, :], c_sl,
                                 mybir.ActivationFunctionType.Tanh)
            nc.vector.tensor_mul(h_t[:, :], o_t[:, :], tanhc[:, :])

            A, B = PAIRS[x]
            if last:
                for q, pl in ((0, A), (1, B)):
                    src3 = h_t[32 * q:32 * q + 32, :].rearrange(
                        "p (y x) -> p y x", y=32, x=32)
                    nc.sync.dma_start(out=hout_d[:, pl, :, :], in_=src3)
                return

            if x == 3:
                # h for planes 0 and 7 -> collective input, then fire the
                # AllGather; its wire time hides under the next step's
                # interior-tile matmuls
                nc.sync.dma_start(out=agin[0], in_=h_t[0:32, :])
                nc.sync.dma_start(out=agin[1], in_=h_t[32:64, :])
                if halo:
                    nc.gpsimd.collective_compute(
                        "AllGather", mybir.AluOpType.bypass, replica_groups=RG,
                        ins=[agin[:, :, :]], outs=[agout[:, :, :]])
                    halo_lo = agout[bass.ds(rv_lo, 1)].squeeze(0).rearrange(
                        "c (y x) -> c y x", y=32, x=32)
                    halo_hi = agout[bass.ds(rv_hi, 1)].squeeze(0).rearrange(
                        "c (y x) -> c y x", y=32, x=32)
                    nc.gpsimd.dma_start(out=nxtv[0:32, 0, 1:33, 1:33],
                                        in_=halo_lo)
                    nc.gpsimd.dma_start(out=nxtv[64:96, 7, 1:33, 1:33],
                                        in_=halo_hi)

            if not bcast:
                return
            h3 = h_t[:, :].rearrange("p (y x) -> p y x", y=32, x=32)
            if B == A + 1:
                # fused broadcasts: [g0<-hA, g1<-hB] at plane A+1, one
                # 64-partition op; [32:96] in one op would cross a quadrant
                # boundary at base 32 (illegal) -> two singles
                bc_rot(nxtv[0:64, A + 1, 1:33, 1:33], h3[0:64])
                bc_rot(nxtv[32:64, A, 1:33, 1:33], h3[0:32])
                bc_rot(nxtv[64:96, A, 1:33, 1:33], h3[32:64])
                if A - 1 >= 0:
                    bc_rot(nxtv[64:96, A - 1, 1:33, 1:33], h3[0:32])
                if B + 1 <= 7:
                    bc_rot(nxtv[0:32, B + 1, 1:33, 1:33], h3[32:64])
            else:
                # pair (0,7): four singles; the two slots only consumed by
                # the next step's (0,7) tile go to gpsimd (it is idle after
                # the collective completes, which is exactly their window)
                bc_copy(nc.gpsimd, nxtv[32:64, 0, 1:33, 1:33], h3[0:32])
                bc_rot(nxtv[0:32, 1, 1:33, 1:33], h3[0:32])
                bc_copy(nc.gpsimd, nxtv[32:64, 7, 1:33, 1:33], h3[32:64])
                bc_rot(nxtv[64:96, 6, 1:33, 1:33], h3[32:64])

        T_ = nsteps
        for t in range(T_):
            curv, nxtv = hsv[t % 2], hsv[(t + 1) % 2]
            nxt = hstack[(t + 1) % 2]
            if t + 1 < T_:
                nc.sync.dma_start(out=nxt[96:124, :], in_=xim_d[t + 1])

            gts = [None] * 4
            for x in range(4):
                if x >= 1:
                    emit_tile_pw(x - 1, gts[x - 1], nxtv, t)
                gts[x] = emit_tile_mm(x, curv, t)
            emit_tile_pw(3, gts[3], nxtv, t)

    nc.finalize()
    _prog_cache[key] = nc
    return nc


def _host_inputs(input_batch, Wx, Wh, b):
    input_batch = np.asarray(input_batch, dtype=np.float32)
    Wx = np.asarray(Wx, dtype=np.float32)
    Wh = np.asarray(Wh, dtype=np.float32)
    b = np.asarray(b, dtype=np.float32)

    import ml_dtypes

    def to_bf16(a):
        return np.asarray(a, np.float32).astype(ml_dtypes.bfloat16)

    xp = np.zeros((2, T, 66, 66, 66), np.float32)
    xp[:, :, 1:65, 1:65, 1:65] = input_batch[:, :, 0]

    whl = np.zeros((9, 128, 128), np.float32)
    for di, (dy, dx) in enumerate(DELTAS):
        for g in range(3):
            whl[di, 32 * g:32 * g + 32, :] = Wh[:, :, g, dy, dx].T
    whl[0, 96:123, :] = Wx[:, 0].reshape(128, 27).T
    whl[0, 123, :] = b

    in_maps = []
    for c in range(8):
        bidx, k = divmod(c, 4)
        z0 = 8 * k
        xim = np.zeros((T, 28, SLAB, PLW, PLW), np.float32)
        for tz in range(3):
            for ty in range(3):
                for tx in range(3):
                    tap = tz * 9 + ty * 3 + tx
                    xim[:, tap, :, 0:32, 0:32] = xp[
                        bidx, :, 2 * z0 + tz:2 * z0 + tz + 16:2,
                        ty:ty + 64:2, tx:tx + 64:2]
        xim[:, 27, :, 0:32, 0:32] = 1.0
        lo_slot = c * 3 + 2 if k == 0 else (c - 1) * 3 + 1
        hi_slot = c * 3 + 2 if k == 3 else (c + 1) * 3 + 0
        in_maps.append({
            "xim": to_bf16(xim.reshape(T, 28, HS_FREE)),
            "whl": to_bf16(whl),
            "hoff": np.array([[lo_slot, hi_slot]], np.int32),
        })
    return in_maps


def run_cores(in_maps, nsteps=T, halo=True, bcast=True, pw=True, **kwargs):
    nc = _build_program(nsteps, halo, bcast, pw)
    return run_bass_kernel_spmd(nc, in_maps, list(range(8)), **kwargs)


def kernel(input_batch, Wx, Wh, b):
    in_maps = _host_inputs(input_batch, Wx, Wh, b)
    res = run_cores(in_maps)
    out = np.zeros((2, CH, 32, 32, 32), np.float32)
    for c in range(8):
        bidx, k = divmod(c, 4)
        out[bidx, :, 8 * k:8 * k + 8] = res.results[c]["hout"]
    return out
